# revision 19
# baseline (speedup 1.0000x reference)
"""Trainium2 Bass kernel for segment_sum (scatter-add of edge features into nodes).

Strategy: 2M edges split contiguously across 8 NeuronCores (250k each).
Host-side prep (layout only, no FP arithmetic): sort each core's edges by
node id, pad every node run to EVEN length (pad slots carry h=0 inside
the run), cut the padded stream into 128 partition streams at run
boundaries, pad each to 2432 slots. The even/odd (A/B) halves of each
pair are laid out across partition halves: tensor T1 holds streams 0-63
(A in partitions 0-63, B in partitions 64-127), T2 holds streams 64-127.

Device (per core), three engines in a pipeline:
  1. Pairing on the PE: two matmuls per 512-column chunk against fixed
     0/1 pairing matrices accumulate A+B for all 128 streams into one
     PSUM tile (M1 fills rows 0-63 from T1, M2 accumulates rows 64-127
     from T2). The stationaries never change, the moving operand is the
     raw edge stream.
  2. ACT copies each PSUM chunk to the SBUF scan buffer (f32 -> bf16).
  3. Segmented scan over the pair-sum stream on the DVE:
     state = m2[t]*state + s[t] (fp32 state, ~2.08 ns/el) per feature
     channel. At the last pair of each node run, state holds that
     node's complete per-core sum.
This removes the pairing adds from the DVE (the wall), leaving it
~86us of pure scans; PE (~50us) and ACT (~55us) hide underneath.
The scan buffer is DMA'd back (bf16); the host picks the run-end pairs
and adds the 8 per-core partials (same unshard-add as the baseline).
"""
import numpy as np
import ml_dtypes

import concourse.bass as bass
import concourse.bacc as bacc
import concourse.mybir as mybir
from concourse import tile
from concourse.bass_utils import run_bass_kernel_spmd

BF16 = mybir.dt.bfloat16
F32 = mybir.dt.float32
OP = mybir.AluOpType

E = 2_000_000
D = 32
N = 100_000
CORES = 8
EPC = E // CORES            # 250_000
PARTS = 128
SLOTS = 2432                # even-run-padded raw slots per partition (max seen ~2350)
HLEN = SLOTS // 2           # 1216 pairs per channel row
FREE = 2 * D * HLEN         # h_in free elements (T1+T2 per channel)
CHUNKS = [(0, 512), (512, 512), (1024, HLEN - 1024)]
# channel groups: DMA-in, PE pairing, ACT copy, scans and DMA-out are
# pipelined per group; first groups small so compute starts early
GROUPS = [1, 1, 2, 4, 4, 4, 4, 4, 4, 4]
WARM = 3                    # first groups pair directly on the DVE
OUT_SPLIT = {9: [1, 1, 1, 1]}  # group idx -> out-DMA sub-splits


def build_program():
    nc = bacc.Bacc("TRN2", target_bir_lowering=False, debug=False,
                   num_devices=CORES)
    h_in = nc.dram_tensor("h", [PARTS, FREE], BF16, kind="ExternalInput")
    m_in = nc.dram_tensor("m", [PARTS, HLEN], BF16, kind="ExternalInput")
    p_in = nc.dram_tensor("p", [PARTS, 2 * PARTS], BF16,
                          kind="ExternalInput")
    s_out = nc.dram_tensor("s", [PARTS, D * HLEN], BF16,
                           kind="ExternalOutput")

    with tile.TileContext(nc) as tc:
        with tc.tile_pool(name="mask", bufs=1) as mp, \
             tc.tile_pool(name="scan", bufs=1) as sp, \
             tc.tile_pool(name="work", bufs=2) as wp, \
             tc.tile_pool(name="ps", bufs=8, space="PSUM") as psp:
            mt = mp.tile([PARTS, HLEN], BF16)
            nc.sync.dma_start(mt[:], m_in[:])
            m1t = mp.tile([PARTS, PARTS], BF16)
            m2t = mp.tile([PARTS, PARTS], BF16)
            nc.sync.dma_start(m1t[:], p_in[:, :PARTS])
            nc.sync.dma_start(m2t[:], p_in[:, PARTS:])
            sc = sp.tile([PARTS, D * HLEN], BF16)

            off = 0
            d = 0
            for gi, ng in enumerate(GROUPS):
                gt = wp.tile([PARTS, ng * 2 * HLEN], BF16, tag="g")
                nc.sync.dma_start(gt[:], h_in[:, off:off + ng * 2 * HLEN])
                if gi < WARM:
                    # warm-up path: pair and scan directly on the DVE in
                    # gt (same-partition A|B layout) while the PE->ACT
                    # pipeline fills for the later groups
                    row = ng * 2 * HLEN
                    a_ap = bass.AP(gt.tensor, 0,
                                   [[row, PARTS], [HLEN, ng], [1, HLEN]])
                    b_ap = bass.AP(gt.tensor, ng * HLEN,
                                   [[row, PARTS], [HLEN, ng], [1, HLEN]])
                    nc.vector.tensor_tensor(a_ap, a_ap, b_ap, OP.add)
                    for ci in range(ng):
                        lo = ci * HLEN
                        nc.vector.tensor_tensor_scan(
                            gt[:, lo:lo + HLEN], mt[:],
                            gt[:, lo:lo + HLEN], 0.0, OP.mult, OP.add)
                        nc.sync.dma_start(
                            s_out[:, (d + ci) * HLEN:(d + ci + 1) * HLEN],
                            gt[:, lo:lo + HLEN])
                    off += ng * 2 * HLEN
                    d += ng
                    continue
                for ci in range(ng):
                    dd = d + ci
                    t1o = ci * HLEN
                    t2o = (ng + ci) * HLEN
                    for co, cn in CHUNKS:
                        ps = psp.tile([PARTS, 512], F32, tag="ps")
                        nc.tensor.matmul(
                            ps[:, :cn], m1t[:], gt[:, t1o + co:t1o + co + cn],
                            start=True, stop=False)
                        nc.tensor.matmul(
                            ps[:, :cn], m2t[:], gt[:, t2o + co:t2o + co + cn],
                            start=False, stop=True)
                        nc.scalar.copy(
                            sc[:, dd * HLEN + co:dd * HLEN + co + cn],
                            ps[:, :cn])
                for ci in range(ng):
                    lo = (d + ci) * HLEN
                    # in-place segmented scan over the pair-sum stream
                    nc.vector.tensor_tensor_scan(
                        sc[:, lo:lo + HLEN], mt[:],
                        sc[:, lo:lo + HLEN], 0.0, OP.mult, OP.add)
                for so, sn in zip(
                        np.cumsum([0] + OUT_SPLIT.get(gi, [ng]))[:-1],
                        OUT_SPLIT.get(gi, [ng])):
                    lo = (d + int(so)) * HLEN
                    nc.sync.dma_start(
                        s_out[:, lo:lo + int(sn) * HLEN],
                        sc[:, lo:lo + int(sn) * HLEN])
                off += ng * 2 * HLEN
                d += ng
    nc.compile()
    return nc


_prog_cache = {}


def _get_prog():
    if "nc" not in _prog_cache:
        _prog_cache["nc"] = build_program()
    return _prog_cache["nc"]


def _pair_matrices():
    m1 = np.zeros((PARTS, PARTS), np.float32)
    m2 = np.zeros((PARTS, PARTS), np.float32)
    for m in range(64):
        m1[m, m] = 1.0
        m1[m + 64, m] = 1.0
        m2[m, m + 64] = 1.0
        m2[m + 64, m + 64] = 1.0
    return np.concatenate([m1, m2], axis=1).astype(ml_dtypes.bfloat16)


def kernel(H, X_node, node_num):
    H = np.ascontiguousarray(np.asarray(H, dtype=np.float32))
    X = np.asarray(X_node).astype(np.int64)
    assert H.shape == (E, D) and X.shape == (E,)
    nc = _get_prog()
    p_dev = _pair_matrices()

    in_maps = []
    metas = []
    for c in range(CORES):
        Xc = X[c * EPC:(c + 1) * EPC]
        Hc = H[c * EPC:(c + 1) * EPC]
        perm = np.argsort(Xc, kind="stable")
        Xs = Xc[perm]
        Hs = Hc[perm]
        # node runs; pad each run to even length (pad slot: h=0, same node)
        runstarts = np.concatenate([[0], np.flatnonzero(np.diff(Xs)) + 1])
        R = len(runstarts)
        L = np.diff(np.concatenate([runstarts, [EPC]]))
        odd = (L & 1).astype(bool)
        start2 = np.concatenate([[0], np.cumsum(L + (L & 1))])
        T = int(start2[-1])
        run_of = np.repeat(np.arange(R), L)
        pos2 = start2[:-1][run_of] + (np.arange(EPC) - runstarts[run_of])
        node2 = np.full(T, -1, np.int64)
        h2 = np.zeros((T, D), np.float32)
        node2[pos2] = Xs
        h2[pos2] = Hs
        node2[start2[1:][odd] - 1] = Xs[runstarts[odd]]

        # cut the padded stream at run boundaries into 128 streams
        tgt = np.arange(1, PARTS) * ((T + PARTS - 1) // PARTS)
        ci = np.minimum(np.searchsorted(start2[:-1], tgt), R - 1)
        cuts = np.concatenate([[0], start2[:-1][ci], [T]])
        cnt = np.diff(cuts)
        assert cnt.max() <= SLOTS, f"partition stream overflow: {cnt.max()}"

        node_pad = np.full((PARTS, SLOTS), -1, np.int64)
        h_pad = np.zeros((PARTS, SLOTS, D), np.float32)
        pidx = np.repeat(np.arange(PARTS), cnt)
        eidx = np.arange(T) - np.repeat(cuts[:-1], cnt)
        node_pad[pidx, eidx] = node2
        h_pad[pidx, eidx] = h2
        m = np.zeros((PARTS, SLOTS), np.float32)
        m[:, 1:] = node_pad[:, 1:] == node_pad[:, :-1]
        m2 = m[:, 0::2]                       # pair-level mask [PARTS, HLEN]

        # T1: streams 0-63 (A rows 0-63, B rows 64-127); T2: streams 64-127
        At = h_pad[:, 0::2, :].transpose(0, 2, 1)   # [PARTS, D, HLEN]
        Bt = h_pad[:, 1::2, :].transpose(0, 2, 1)
        T1 = np.concatenate([At[:64], Bt[:64]], axis=0)
        T2 = np.concatenate([At[64:], Bt[64:]], axis=0)
        blocks = []
        d0 = 0
        for gi, ng in enumerate(GROUPS):
            if gi < WARM:
                blocks.append(At[:, d0:d0 + ng, :].reshape(PARTS, -1))
                blocks.append(Bt[:, d0:d0 + ng, :].reshape(PARTS, -1))
            else:
                blocks.append(T1[:, d0:d0 + ng, :].reshape(PARTS, -1))
                blocks.append(T2[:, d0:d0 + ng, :].reshape(PARTS, -1))
            d0 += ng
        h_dev = np.concatenate(blocks, axis=1).astype(ml_dtypes.bfloat16)
        m_dev = np.ascontiguousarray(m2).astype(ml_dtypes.bfloat16)
        in_maps.append({"h": np.ascontiguousarray(h_dev), "m": m_dev,
                        "p": p_dev})
        metas.append(node_pad[:, 0::2])       # node id per pair

    _prog_cache["last_inputs"] = in_maps
    # The very first execution of a freshly loaded program has been
    # observed (once) to return corrupted results; correct runs are
    # bit-identical. Run until two consecutive executions agree.
    res = run_bass_kernel_spmd(nc, in_maps, core_ids=list(range(CORES)),
                               trace=False)
    for _ in range(3):
        res2 = run_bass_kernel_spmd(nc, in_maps, core_ids=list(range(CORES)),
                                    trace=False)
        if all(
            np.array_equal(
                res.results[c]["s"].view(np.uint16),
                res2.results[c]["s"].view(np.uint16))
            for c in range(CORES)
        ):
            break
        res = res2

    out = np.zeros((N, D), np.float32)
    for c in range(CORES):
        node_pair = metas[c]                  # [PARTS, HLEN]
        s = np.asarray(res.results[c]["s"]).astype(np.float32)
        s = s.reshape(PARTS, D, HLEN)
        nxt = np.concatenate(
            [node_pair[:, 1:], np.full((PARTS, 1), -2, np.int64)], axis=1)
        is_end = (node_pair >= 0) & (node_pair != nxt)
        pp, ii = np.nonzero(is_end)
        nodes = node_pair[pp, ii]
        vals = s[pp, :, ii]
        # within one core each node has exactly one run end -> unique idx
        out[nodes] += vals
    return out


# revision 20
# speedup vs baseline: 1.1823x; 1.1823x over previous
"""Trainium2 Bass kernel for segment_sum (scatter-add of edge features into nodes).

Strategy: 2M edges split contiguously across 8 NeuronCores (250k each).
Host-side prep (layout only, no FP arithmetic): sort each core's edges by
node id, pad every node run to EVEN length (pad slots carry h=0 inside
the run), cut the padded stream into 128 partition streams at run
boundaries, pad each to 2432 slots. The even/odd (A/B) halves of each
pair are laid out across partition halves: tensor T1 holds streams 0-63
(A in partitions 0-63, B in partitions 64-127), T2 holds streams 64-127.

Device (per core), three engines in a pipeline:
  1. Pairing on the PE: two matmuls per 512-column chunk against fixed
     0/1 pairing matrices accumulate A+B for all 128 streams into one
     PSUM tile (M1 fills rows 0-63 from T1, M2 accumulates rows 64-127
     from T2). The stationaries never change, the moving operand is the
     raw edge stream.
  2. ACT copies each PSUM chunk to the SBUF scan buffer (f32 -> bf16).
  3. Segmented scan over the pair-sum stream on the DVE:
     state = m2[t]*state + s[t] (fp32 state, ~2.08 ns/el) per feature
     channel. At the last pair of each node run, state holds that
     node's complete per-core sum.
This removes the pairing adds from the DVE (the wall), leaving it
~86us of pure scans; PE (~50us) and ACT (~55us) hide underneath.
The scan buffer is DMA'd back (bf16); the host picks the run-end pairs
and adds the 8 per-core partials (same unshard-add as the baseline).
"""
import numpy as np
import ml_dtypes

import concourse.bass as bass
import concourse.bacc as bacc
import concourse.mybir as mybir
from concourse import tile
from concourse.bass_utils import run_bass_kernel_spmd

BF16 = mybir.dt.bfloat16
F32 = mybir.dt.float32
OP = mybir.AluOpType

E = 2_000_000
D = 32
N = 100_000
CORES = 8
EPC = E // CORES            # 250_000
PARTS = 128
SLOTS = 2432                # even-run-padded raw slots per partition (max seen ~2350)
HLEN = SLOTS // 2           # 1216 pairs per channel row
FREE = 2 * D * HLEN         # h_in free elements (T1+T2 per channel)
CHUNKS = [(0, 512), (512, 512), (1024, HLEN - 1024)]
# channel groups: DMA-in, PE pairing, ACT copy, scans and DMA-out are
# pipelined per group; first groups small so compute starts early
GROUPS = [1, 1, 2, 4, 4, 4, 4, 4, 4, 4]
OUT_SPLIT = {9: [2, 1, 1]}  # group idx -> out-DMA sub-splits


def build_program():
    nc = bacc.Bacc("TRN2", target_bir_lowering=False, debug=False,
                   num_devices=CORES)
    h_in = nc.dram_tensor("h", [PARTS, FREE], BF16, kind="ExternalInput")
    m_in = nc.dram_tensor("m", [PARTS, HLEN], BF16, kind="ExternalInput")
    p_in = nc.dram_tensor("p", [PARTS, 2 * PARTS], BF16,
                          kind="ExternalInput")
    s_out = nc.dram_tensor("s", [PARTS, D * HLEN], BF16,
                           kind="ExternalOutput")

    with tile.TileContext(nc) as tc:
        with tc.tile_pool(name="mask", bufs=1) as mp, \
             tc.tile_pool(name="scan", bufs=1) as sp, \
             tc.tile_pool(name="work", bufs=2) as wp, \
             tc.tile_pool(name="ps", bufs=6, space="PSUM") as psp:
            mt = mp.tile([PARTS, HLEN], BF16)
            nc.sync.dma_start(mt[:], m_in[:])
            m1t = mp.tile([PARTS, PARTS], BF16)
            m2t = mp.tile([PARTS, PARTS], BF16)
            nc.sync.dma_start(m1t[:], p_in[:, :PARTS])
            nc.sync.dma_start(m2t[:], p_in[:, PARTS:])
            sc = sp.tile([PARTS, D * HLEN], BF16)

            off = 0
            d = 0
            for gi, ng in enumerate(GROUPS):
                gt = wp.tile([PARTS, ng * 2 * HLEN], BF16, tag="g")
                nc.sync.dma_start(gt[:], h_in[:, off:off + ng * 2 * HLEN])
                for ci in range(ng):
                    dd = d + ci
                    t1o = ci * HLEN
                    t2o = (ng + ci) * HLEN
                    for co, cn in CHUNKS:
                        ps = psp.tile([PARTS, 512], F32, tag="ps")
                        nc.tensor.matmul(
                            ps[:, :cn], m1t[:], gt[:, t1o + co:t1o + co + cn],
                            start=True, stop=False)
                        nc.tensor.matmul(
                            ps[:, :cn], m2t[:], gt[:, t2o + co:t2o + co + cn],
                            start=False, stop=True)
                        nc.scalar.copy(
                            sc[:, dd * HLEN + co:dd * HLEN + co + cn],
                            ps[:, :cn])
                for ci in range(ng):
                    lo = (d + ci) * HLEN
                    # in-place segmented scan over the pair-sum stream
                    nc.vector.tensor_tensor_scan(
                        sc[:, lo:lo + HLEN], mt[:],
                        sc[:, lo:lo + HLEN], 0.0, OP.mult, OP.add)
                for so, sn in zip(
                        np.cumsum([0] + OUT_SPLIT.get(gi, [ng]))[:-1],
                        OUT_SPLIT.get(gi, [ng])):
                    lo = (d + int(so)) * HLEN
                    nc.sync.dma_start(
                        s_out[:, lo:lo + int(sn) * HLEN],
                        sc[:, lo:lo + int(sn) * HLEN])
                off += ng * 2 * HLEN
                d += ng
    nc.compile()
    return nc


_prog_cache = {}


def _get_prog():
    if "nc" not in _prog_cache:
        _prog_cache["nc"] = build_program()
    return _prog_cache["nc"]


def _pair_matrices():
    m1 = np.zeros((PARTS, PARTS), np.float32)
    m2 = np.zeros((PARTS, PARTS), np.float32)
    for m in range(64):
        m1[m, m] = 1.0
        m1[m + 64, m] = 1.0
        m2[m, m + 64] = 1.0
        m2[m + 64, m + 64] = 1.0
    return np.concatenate([m1, m2], axis=1).astype(ml_dtypes.bfloat16)


def kernel(H, X_node, node_num):
    H = np.ascontiguousarray(np.asarray(H, dtype=np.float32))
    X = np.asarray(X_node).astype(np.int64)
    assert H.shape == (E, D) and X.shape == (E,)
    nc = _get_prog()
    p_dev = _pair_matrices()

    in_maps = []
    metas = []
    for c in range(CORES):
        Xc = X[c * EPC:(c + 1) * EPC]
        Hc = H[c * EPC:(c + 1) * EPC]
        perm = np.argsort(Xc, kind="stable")
        Xs = Xc[perm]
        Hs = Hc[perm]
        # node runs; pad each run to even length (pad slot: h=0, same node)
        runstarts = np.concatenate([[0], np.flatnonzero(np.diff(Xs)) + 1])
        R = len(runstarts)
        L = np.diff(np.concatenate([runstarts, [EPC]]))
        odd = (L & 1).astype(bool)
        start2 = np.concatenate([[0], np.cumsum(L + (L & 1))])
        T = int(start2[-1])
        run_of = np.repeat(np.arange(R), L)
        pos2 = start2[:-1][run_of] + (np.arange(EPC) - runstarts[run_of])
        node2 = np.full(T, -1, np.int64)
        h2 = np.zeros((T, D), np.float32)
        node2[pos2] = Xs
        h2[pos2] = Hs
        node2[start2[1:][odd] - 1] = Xs[runstarts[odd]]

        # cut the padded stream at run boundaries into 128 streams
        tgt = np.arange(1, PARTS) * ((T + PARTS - 1) // PARTS)
        ci = np.minimum(np.searchsorted(start2[:-1], tgt), R - 1)
        cuts = np.concatenate([[0], start2[:-1][ci], [T]])
        cnt = np.diff(cuts)
        assert cnt.max() <= SLOTS, f"partition stream overflow: {cnt.max()}"

        node_pad = np.full((PARTS, SLOTS), -1, np.int64)
        h_pad = np.zeros((PARTS, SLOTS, D), np.float32)
        pidx = np.repeat(np.arange(PARTS), cnt)
        eidx = np.arange(T) - np.repeat(cuts[:-1], cnt)
        node_pad[pidx, eidx] = node2
        h_pad[pidx, eidx] = h2
        m = np.zeros((PARTS, SLOTS), np.float32)
        m[:, 1:] = node_pad[:, 1:] == node_pad[:, :-1]
        m2 = m[:, 0::2]                       # pair-level mask [PARTS, HLEN]

        # T1: streams 0-63 (A rows 0-63, B rows 64-127); T2: streams 64-127
        At = h_pad[:, 0::2, :].transpose(0, 2, 1)   # [PARTS, D, HLEN]
        Bt = h_pad[:, 1::2, :].transpose(0, 2, 1)
        T1 = np.concatenate([At[:64], Bt[:64]], axis=0)
        T2 = np.concatenate([At[64:], Bt[64:]], axis=0)
        blocks = []
        d0 = 0
        for ng in GROUPS:
            blocks.append(T1[:, d0:d0 + ng, :].reshape(PARTS, -1))
            blocks.append(T2[:, d0:d0 + ng, :].reshape(PARTS, -1))
            d0 += ng
        h_dev = np.concatenate(blocks, axis=1).astype(ml_dtypes.bfloat16)
        m_dev = np.ascontiguousarray(m2).astype(ml_dtypes.bfloat16)
        in_maps.append({"h": np.ascontiguousarray(h_dev), "m": m_dev,
                        "p": p_dev})
        metas.append(node_pad[:, 0::2])       # node id per pair

    _prog_cache["last_inputs"] = in_maps
    # The very first execution of a freshly loaded program has been
    # observed (once) to return corrupted results; correct runs are
    # bit-identical. Run until two consecutive executions agree.
    res = run_bass_kernel_spmd(nc, in_maps, core_ids=list(range(CORES)),
                               trace=False)
    for _ in range(3):
        res2 = run_bass_kernel_spmd(nc, in_maps, core_ids=list(range(CORES)),
                                    trace=False)
        if all(
            np.array_equal(
                res.results[c]["s"].view(np.uint16),
                res2.results[c]["s"].view(np.uint16))
            for c in range(CORES)
        ):
            break
        res = res2

    out = np.zeros((N, D), np.float32)
    for c in range(CORES):
        node_pair = metas[c]                  # [PARTS, HLEN]
        s = np.asarray(res.results[c]["s"]).astype(np.float32)
        s = s.reshape(PARTS, D, HLEN)
        nxt = np.concatenate(
            [node_pair[:, 1:], np.full((PARTS, 1), -2, np.int64)], axis=1)
        is_end = (node_pair >= 0) & (node_pair != nxt)
        pp, ii = np.nonzero(is_end)
        nodes = node_pair[pp, ii]
        vals = s[pp, :, ii]
        # within one core each node has exactly one run end -> unique idx
        out[nodes] += vals
    return out


# revision 21
# speedup vs baseline: 1.2674x; 1.0720x over previous
"""Trainium2 Bass kernel for segment_sum (scatter-add of edge features into nodes).

Strategy: 2M edges split contiguously across 8 NeuronCores (250k each).
Host-side prep (layout only, no FP arithmetic): sort each core's edges by
node id, pad every node run to EVEN length (pad slots carry h=0 inside
the run), cut the padded stream into 128 partition streams at run
boundaries, pad each to 2432 slots. The even/odd (A/B) halves of each
pair are laid out across partition halves: tensor T1 holds streams 0-63
(A in partitions 0-63, B in partitions 64-127), T2 holds streams 64-127.

Device (per core), three engines in a pipeline:
  1. Pairing on the PE: two matmuls per 512-column chunk against fixed
     0/1 pairing matrices accumulate A+B for all 128 streams into one
     PSUM tile (M1 fills rows 0-63 from T1, M2 accumulates rows 64-127
     from T2). The stationaries never change, the moving operand is the
     raw edge stream.
  2. ACT copies each PSUM chunk to the SBUF scan buffer (f32 -> bf16).
  3. Segmented scan over the pair-sum stream on the DVE:
     state = m2[t]*state + s[t] (fp32 state, ~2.08 ns/el) per feature
     channel. At the last pair of each node run, state holds that
     node's complete per-core sum.
This removes the pairing adds from the DVE (the wall), leaving it
~86us of pure scans; PE (~50us) and ACT (~55us) hide underneath.
The scan buffer is DMA'd back (bf16); the host picks the run-end pairs
and adds the 8 per-core partials (same unshard-add as the baseline).
"""
import numpy as np
import ml_dtypes

import concourse.bass as bass
import concourse.bacc as bacc
import concourse.mybir as mybir
from concourse import tile
from concourse.bass_utils import run_bass_kernel_spmd

BF16 = mybir.dt.bfloat16
F32 = mybir.dt.float32
OP = mybir.AluOpType

E = 2_000_000
D = 32
N = 100_000
CORES = 8
EPC = E // CORES            # 250_000
PARTS = 128
SLOTS = 2432                # even-run-padded raw slots per partition (max seen ~2350)
HLEN = SLOTS // 2           # 1216 pairs per channel row
FREE = 2 * D * HLEN         # h_in free elements (T1+T2 per channel)
CHUNKS = [(0, 512), (512, 512), (1024, HLEN - 1024)]
# channel groups: DMA-in, PE pairing, ACT copy, scans and DMA-out are
# pipelined per group; first groups small so compute starts early
GROUPS = [1, 1, 2, 4, 4, 4, 4, 4, 4, 4]
OUT_SPLIT = {9: [2, 1, 1]}  # group idx -> out-DMA sub-splits


def build_program():
    nc = bacc.Bacc("TRN2", target_bir_lowering=False, debug=False,
                   num_devices=CORES)
    h_in = nc.dram_tensor("h", [PARTS, FREE], BF16, kind="ExternalInput")
    m_in = nc.dram_tensor("m", [PARTS, HLEN], BF16, kind="ExternalInput")
    p_in = nc.dram_tensor("p", [PARTS, 2 * PARTS], BF16,
                          kind="ExternalInput")
    s_out = nc.dram_tensor("s", [PARTS, D * HLEN], BF16,
                           kind="ExternalOutput")

    with tile.TileContext(nc) as tc:
        with tc.tile_pool(name="mask", bufs=1) as mp, \
             tc.tile_pool(name="scan", bufs=1) as sp, \
             tc.tile_pool(name="work", bufs=3) as wp, \
             tc.tile_pool(name="ps", bufs=8, space="PSUM") as psp:
            mt = mp.tile([PARTS, HLEN], BF16)
            nc.sync.dma_start(mt[:], m_in[:])
            m1t = mp.tile([PARTS, PARTS], BF16)
            m2t = mp.tile([PARTS, PARTS], BF16)
            nc.sync.dma_start(m1t[:], p_in[:, :PARTS])
            nc.sync.dma_start(m2t[:], p_in[:, PARTS:])
            sc = sp.tile([PARTS, D * HLEN], BF16)

            off = 0
            d = 0
            for gi, ng in enumerate(GROUPS):
                gt = wp.tile([PARTS, ng * 2 * HLEN], BF16, tag="g")
                nc.sync.dma_start(gt[:], h_in[:, off:off + ng * 2 * HLEN])
                for ci in range(ng):
                    dd = d + ci
                    t1o = ci * HLEN
                    t2o = (ng + ci) * HLEN
                    for co, cn in CHUNKS:
                        ps = psp.tile([PARTS, 512], F32, tag="ps")
                        nc.tensor.matmul(
                            ps[:, :cn], m1t[:], gt[:, t1o + co:t1o + co + cn],
                            start=True, stop=False)
                        nc.tensor.matmul(
                            ps[:, :cn], m2t[:], gt[:, t2o + co:t2o + co + cn],
                            start=False, stop=True)
                        nc.scalar.copy(
                            sc[:, dd * HLEN + co:dd * HLEN + co + cn],
                            ps[:, :cn])
                for ci in range(ng):
                    lo = (d + ci) * HLEN
                    # in-place segmented scan over the pair-sum stream
                    nc.vector.tensor_tensor_scan(
                        sc[:, lo:lo + HLEN], mt[:],
                        sc[:, lo:lo + HLEN], 0.0, OP.mult, OP.add)
                for so, sn in zip(
                        np.cumsum([0] + OUT_SPLIT.get(gi, [ng]))[:-1],
                        OUT_SPLIT.get(gi, [ng])):
                    lo = (d + int(so)) * HLEN
                    nc.sync.dma_start(
                        s_out[:, lo:lo + int(sn) * HLEN],
                        sc[:, lo:lo + int(sn) * HLEN])
                off += ng * 2 * HLEN
                d += ng
    nc.compile()
    return nc


_prog_cache = {}


def _get_prog():
    if "nc" not in _prog_cache:
        _prog_cache["nc"] = build_program()
    return _prog_cache["nc"]


def _pair_matrices():
    m1 = np.zeros((PARTS, PARTS), np.float32)
    m2 = np.zeros((PARTS, PARTS), np.float32)
    for m in range(64):
        m1[m, m] = 1.0
        m1[m + 64, m] = 1.0
        m2[m, m + 64] = 1.0
        m2[m + 64, m + 64] = 1.0
    return np.concatenate([m1, m2], axis=1).astype(ml_dtypes.bfloat16)


def kernel(H, X_node, node_num):
    H = np.ascontiguousarray(np.asarray(H, dtype=np.float32))
    X = np.asarray(X_node).astype(np.int64)
    assert H.shape == (E, D) and X.shape == (E,)
    nc = _get_prog()
    p_dev = _pair_matrices()

    in_maps = []
    metas = []
    for c in range(CORES):
        Xc = X[c * EPC:(c + 1) * EPC]
        Hc = H[c * EPC:(c + 1) * EPC]
        perm = np.argsort(Xc, kind="stable")
        Xs = Xc[perm]
        Hs = Hc[perm]
        # node runs; pad each run to even length (pad slot: h=0, same node)
        runstarts = np.concatenate([[0], np.flatnonzero(np.diff(Xs)) + 1])
        R = len(runstarts)
        L = np.diff(np.concatenate([runstarts, [EPC]]))
        odd = (L & 1).astype(bool)
        start2 = np.concatenate([[0], np.cumsum(L + (L & 1))])
        T = int(start2[-1])
        run_of = np.repeat(np.arange(R), L)
        pos2 = start2[:-1][run_of] + (np.arange(EPC) - runstarts[run_of])
        node2 = np.full(T, -1, np.int64)
        h2 = np.zeros((T, D), np.float32)
        node2[pos2] = Xs
        h2[pos2] = Hs
        node2[start2[1:][odd] - 1] = Xs[runstarts[odd]]

        # cut the padded stream at run boundaries into 128 streams
        tgt = np.arange(1, PARTS) * ((T + PARTS - 1) // PARTS)
        ci = np.minimum(np.searchsorted(start2[:-1], tgt), R - 1)
        cuts = np.concatenate([[0], start2[:-1][ci], [T]])
        cnt = np.diff(cuts)
        assert cnt.max() <= SLOTS, f"partition stream overflow: {cnt.max()}"

        node_pad = np.full((PARTS, SLOTS), -1, np.int64)
        h_pad = np.zeros((PARTS, SLOTS, D), np.float32)
        pidx = np.repeat(np.arange(PARTS), cnt)
        eidx = np.arange(T) - np.repeat(cuts[:-1], cnt)
        node_pad[pidx, eidx] = node2
        h_pad[pidx, eidx] = h2
        m = np.zeros((PARTS, SLOTS), np.float32)
        m[:, 1:] = node_pad[:, 1:] == node_pad[:, :-1]
        m2 = m[:, 0::2]                       # pair-level mask [PARTS, HLEN]

        # T1: streams 0-63 (A rows 0-63, B rows 64-127); T2: streams 64-127
        At = h_pad[:, 0::2, :].transpose(0, 2, 1)   # [PARTS, D, HLEN]
        Bt = h_pad[:, 1::2, :].transpose(0, 2, 1)
        T1 = np.concatenate([At[:64], Bt[:64]], axis=0)
        T2 = np.concatenate([At[64:], Bt[64:]], axis=0)
        blocks = []
        d0 = 0
        for ng in GROUPS:
            blocks.append(T1[:, d0:d0 + ng, :].reshape(PARTS, -1))
            blocks.append(T2[:, d0:d0 + ng, :].reshape(PARTS, -1))
            d0 += ng
        h_dev = np.concatenate(blocks, axis=1).astype(ml_dtypes.bfloat16)
        m_dev = np.ascontiguousarray(m2).astype(ml_dtypes.bfloat16)
        in_maps.append({"h": np.ascontiguousarray(h_dev), "m": m_dev,
                        "p": p_dev})
        metas.append(node_pad[:, 0::2])       # node id per pair

    _prog_cache["last_inputs"] = in_maps
    # The very first execution of a freshly loaded program has been
    # observed (once) to return corrupted results; correct runs are
    # bit-identical. Run until two consecutive executions agree.
    res = run_bass_kernel_spmd(nc, in_maps, core_ids=list(range(CORES)),
                               trace=False)
    for _ in range(3):
        res2 = run_bass_kernel_spmd(nc, in_maps, core_ids=list(range(CORES)),
                                    trace=False)
        if all(
            np.array_equal(
                res.results[c]["s"].view(np.uint16),
                res2.results[c]["s"].view(np.uint16))
            for c in range(CORES)
        ):
            break
        res = res2

    out = np.zeros((N, D), np.float32)
    for c in range(CORES):
        node_pair = metas[c]                  # [PARTS, HLEN]
        s = np.asarray(res.results[c]["s"]).astype(np.float32)
        s = s.reshape(PARTS, D, HLEN)
        nxt = np.concatenate(
            [node_pair[:, 1:], np.full((PARTS, 1), -2, np.int64)], axis=1)
        is_end = (node_pair >= 0) & (node_pair != nxt)
        pp, ii = np.nonzero(is_end)
        nodes = node_pair[pp, ii]
        vals = s[pp, :, ii]
        # within one core each node has exactly one run end -> unique idx
        out[nodes] += vals
    return out
